# revision 1
# baseline (speedup 1.0000x reference)
"""Trainium2 Bass kernel for nn_Decoder (input-feeding GRU decoder with Luong attention).

Self-contained: hardcodes shapes B=64, T_DEC=128, T_ENC=256, H=D_X=N_FEED=1024, 8 cores.

Strategy (per core c of 8):
  - Model-parallel GRU/FFN: core c owns the 128-wide hidden slice S_c = [128c, 128c+128)
    of every weight matrix output (Wfeed/Wx/Wh/Watt out-sharded; weights SBUF-resident).
  - Batch-sharded attention: core c owns batch rows [8c, 8c+8) with its o_enc slices
    resident in SBUF (scores operand fp32 transposed; value operand bf16).
  - Per-step cross-core exchanges of [128, 64] fp32 tiles (feedT / h_newT / valueT /
    h_attT) via remote_dma_broadcast (SBUF->SBUF, all 8 cores, runtime landing offset
    = partition_id so blocks land in natural order). Manual counting semaphores.
"""
import sys

sys.path.insert(0, "/opt/trn_rl_repo")

import numpy as np
import ml_dtypes

import concourse.bass as bass
import concourse.bacc as bacc
import concourse.mybir as mybir
from concourse import library_config

FP32 = mybir.dt.float32
BF16 = mybir.dt.bfloat16
FP16 = mybir.dt.float16
AF = mybir.ActivationFunctionType
ALU = mybir.AluOpType
AX = mybir.AxisListType

NCORES = 8
B, T, TE, H, DX = 64, 128, 256, 1024, 1024
BC = B // NCORES      # 8 batch rows per core
HC = H // 128         # 8 hidden chunks
LSEM_INC = 32         # local_sem increment per remote_dma_broadcast (empirical)


def build(nc, steps=T, strict_lsem=False, loopback=False):
    # ---------------- DRAM I/O ----------------
    d_wfeed = nc.declare_dram_parameter("wfeed", [H, 128], FP32, isOutput=False)
    d_wx = nc.declare_dram_parameter("wx", [2 * H, 384], FP32, isOutput=False)
    d_wh = nc.declare_dram_parameter("wh", [H, 384], FP32, isOutput=False)
    d_watt = nc.declare_dram_parameter("watt", [2 * H, 128], FP32, isOutput=False)
    d_oencT = nc.declare_dram_parameter("oencT", [H, BC * TE], FP32, isOutput=False)
    d_oenc = nc.declare_dram_parameter("oenc", [BC * TE, H], FP32, isOutput=False)
    d_xT = nc.declare_dram_parameter("xT", [T, 128, HC * 64], FP32, isOutput=False)
    d_h0T = nc.declare_dram_parameter("h0T", [H, B], FP32, isOutput=False)
    d_h0own = nc.declare_dram_parameter("h0own", [B, 128], FP32, isOutput=False)
    d_ident = nc.declare_dram_parameter("ident", [128, 64], FP32, isOutput=False)
    d_out = nc.declare_dram_parameter("out", [B, T, 128], FP32, isOutput=True)

    # ---------------- SBUF ----------------
    A = nc.alloc_sbuf_tensor
    wfeed = A("wfeed_sb", [128, HC * 128], FP32)
    wx = A("wx_sb", [128, 16 * 384], FP32)
    wh = A("wh_sb", [128, HC * 384], FP32)
    watt = A("watt_sb", [128, 16 * 128], FP32)
    oencT = A("oencT_sb", [128, HC * BC * TE], FP32)     # (hc, b, t)
    oenc = A("oenc_sb", [128, BC * 2 * 1024], FP32)      # (b, tc, h)
    xt = A("xt_sb", [128, HC * 64], FP32)                # (hc, b)
    ident = A("ident_sb", [128, 64], FP32)
    land_F = A("land_F", [128, 512], FP32)
    land_H = A("land_H", [128, 2, 512], FP32)
    land_V = A("land_V", [128, 512], FP32)
    land_A = A("land_A", [128, 2, 512], FP32)
    send_F = A("send_F", [128, 64], FP32)
    send_H = A("send_H", [128, 64], FP32)
    send_V = A("send_V", [128, 64], FP32)
    send_A = A("send_A", [128, 64], FP32)
    gp1 = A("gp1", [128, 128], FP32)   # rb | zb
    gp2 = A("gp2", [128, 128], FP32)   # t1 | t2
    gp3 = A("gp3", [128, 128], FP32)   # hcand | db
    gp5 = A("gp5", [128, 128], FP32)   # feed_s | h_att
    h_new = A("h_new_sb", [64, 2, 128], FP32)
    zd = A("zd_sb", [64, 128], FP32)
    rb, zb = gp1[0:64, :], gp1[64:128, :]
    t1, t2 = gp2[0:64, :], gp2[64:128, :]
    hcand, db = gp3[0:64, :], gp3[64:128, :]
    feed_s = gp5[0:64, :]
    hsc = A("hsc", [128, 64], FP32)                      # own-batch h_newT cols (hc, i)
    scratch = A("scratch", [128, 2048], FP32)            # phased: sc|exp -> v_st -> wv_re
    mx_st = A("mx_st", [128, 2], FP32)
    negmx_st = A("negmx_st", [128, 2], FP32)
    sume_st = A("sume_st", [128, 2], FP32)
    rsum_st = A("rsum_st", [128, 2], FP32)
    attnT = A("attnT", [128, 16], FP32)                  # (tc, b)
    value = A("value_sb", [BC, 1024], FP32)
    attn = value[0:BC, 0:TE]   # overlay: attn dead before value rows are gathered
    h0own = scratch[0:64, 1024:1152]   # only live through step 0's gates
    sc_st = scratch[:, 0:512]
    exp_st = scratch[:, 512:1024]
    attn_st = scratch[:, 0:512]
    v_st = scratch[:, 0:2048]
    wv_re = scratch[:, 0:512]

    # ---------------- PSUM ----------------
    P = nc.place_psum_tensor
    ps_zr = P("ps_zr", [64, 512], FP32, 0)       # z|r 0:256 | hh 256:384
    ps_b1 = P("ps_b1", [64, 512], FP32, 1)       # xh 0:128 | feed 256:384 | att 384:512
    ps_sc = P("ps_sc", [128, 512], FP32, 2)      # scores: round r cols 256r, rows {32g}
    ps_v0 = P("ps_v0", [128, 1024], FP32, 3)     # value round 0 (banks 3-4)
    ps_v1 = P("ps_v1", [128, 1024], FP32, 5)     # value round 1 (banks 5-6)
    ps_tr = P("ps_tr", [128, 512], FP32, 7)      # feedT 0:64 | hT 64:128 | attnT 128:144 | vT 144:208 | hattT 208:272

    # ---------------- semaphores ----------------
    sem = {n: nc.alloc_semaphore(n) for n in [
        "rsF", "rsH", "rsV", "rsA", "lsF", "lsH", "lsV", "lsA", "psem",
        "s_pre",      # prologue DMAs
        "s_xt",       # xt loads (16/load)
        "s_feedp",    # PE feed psum ready
        "s_feedt",    # ACT tanh(feed) done
        "s_ftr",      # PE feedT transpose done
        "s_fsend",    # DVE send_F written
        "s_zr",       # PE zr psum ready
        "s_h2",       # PE xh+hh psum ready
        "s_sigz", "s_zrdone",
        "s_t2",       # DVE t2 ready
        "s_hcand",    # ACT hcand done
        "s_gates",    # DVE h_new written
        "s_htr",      # PE hT transpose done
        "s_hsend",    # DVE send_H written
        "s_hscf",     # sync h_scores extract done (16)
        "s_sc",       # PE scores done
        "s_sc8",      # sync scores gather done (16)
        "s_scst",     # DVE scores psum->sbuf copy done
        "s_attng",    # sync attn gather done (16)
        "s_vst",      # ACT value psum->sbuf copies done
        "s_negmx",    # DVE negmax ready
        "s_exp",      # ACT exp done
        "s_attn",     # DVE attn normalized
        "s_attr",     # PE attnT transposes done
        "s_attnT",    # DVE attnT cast done
        "s_v",        # PE value MMs done
        "s_vx",       # sync value extract done (16)
        "s_vtr",      # PE valueT transposes done
        "s_vsend",    # DVE send_V written
        "s_wvre",     # DVE land_V repack done
        "s_att",      # PE watt MMs done
        "s_hatt",     # ACT tanh(h_att) done
        "s_atr",      # PE hattT transpose done
        "s_asend",    # DVE send_A written
        "s_out",      # output stores (16)
    ]}

    pe, act, dve, gp, sy = nc.tensor, nc.scalar, nc.vector, nc.gpsimd, nc.sync
    # ---------------- start: barrier + library ----------------
    nc.all_core_barrier()
    gp.load_library(library_config.remote_dma)
    my_gp = gp.partition_id()
    my_sy = sy.partition_id()
    my_sc = act.partition_id()

    # ---------------- prologue loads (sync engine) ----------------
    npre = 0

    def pre(dst, src):
        nonlocal npre
        sy.dma_start(out=dst, in_=src).then_inc(sem["s_pre"], 16)
        npre += 16

    # weight-style loads: DRAM [R, C] (R = nchunk*128) -> SBUF [128, nchunk*C]
    def chunked(dst, src, nchunk, ccols):
        s3 = src.rearrange("(j p) c -> p j c", p=128)
        d3 = dst[:].rearrange("p (j c) -> p j c", c=ccols)
        pre(d3, s3)

    def chunked_ap(dst_ap, src, nchunk, ccols):
        s3 = src.rearrange("(j p) c -> p j c", p=128)
        d3 = dst_ap.rearrange("p (j c) -> p j c", c=ccols)
        pre(d3, s3)

    chunked(wfeed, d_wfeed, HC, 128)
    chunked(wx, d_wx, 16, 384)
    chunked(wh, d_wh, HC, 384)
    chunked(watt, d_watt, 16, 128)
    chunked(oencT, d_oencT, HC, BC * TE)
    chunked(oenc, d_oenc, BC * 2, 1024)
    chunked_ap(land_H[:, 1, :], d_h0T, HC, 64)
    pre(ident[:], d_ident[:])
    pre(h0own, d_h0own[:])
    # xT step-0 load (contiguous [128, 512] rows)
    pre(xt[:], d_xT[0])

    # zero-init col-tiled psum banks (their unused rows are read by bulk copies)
    dve.memset(ps_sc[:], 0.0)
    dve.memset(ps_v0[:], 0.0)
    dve.memset(ps_v1[:], 0.0)
    dve.drain()

    # wait points for prologue
    PRE = npre

    RS = 2 if loopback else 16
    # python-side send counters per exchange
    cnt = {"F": 0, "H": 0, "V": 0, "A": 0}
    nprep = 0

    lands = {"F": land_F, "H": land_H, "V": land_V, "A": land_A}
    sends = {"F": send_F, "H": send_H, "V": send_V, "A": send_A}

    def prep(e, k):
        nonlocal nprep
        par = k % 2
        if e in ("F", "V"):
            out_ap = lands[e][:, bass.ts(my_gp, 64)]
        else:
            out_ap = lands[e][:, par, :][:, bass.ts(my_gp, 64)]
        gp.remote_dma_broadcast(
            out_ap=out_ap,
            in_ap=sends[e][:],
            remote_sem=sem["rs" + e],
            local_sem=sem["ls" + e],
            rdests=([(0, 0)] + [None] * 7) if loopback else [(0, j) for j in range(NCORES)],
        ).then_inc(sem["psem"], 1)
        nprep += 1
        return nprep

    def fire(e, prep_no, ready_sem, ready_val):
        gp.wait_ge(sem["psem"], prep_no)
        gp.wait_ge(sem[ready_sem], ready_val)
        gp.trigger_dma(count=1)
        cnt[e] += 1

    ident64 = ident[0:64, 0:64]
    ident64b = ident[64:128, 0:64]
    ident8 = ident[0:8, 0:8]

    # ================= main loop (fully unrolled) =================
    for k in range(steps):
        par = k % 2
        prv = (k - 1) % 2

        # ---- gpsimd: emit preps for this step's exchanges (descgen ahead) ----
        pF = prep("F", k) if k >= 1 else None
        pH = prep("H", k)
        pV = prep("V", k)
        pA = prep("A", k) if k < steps - 1 else None

        # ---- sync: prefetch xT(k+1) into slot (k+1)%2 ----
        if k + 1 < steps:
            sy.wait_ge(sem["s_h2"], k + 1)  # PE finished x-MMs of step k (single buffer)
            sy.dma_start(out=xt[:], in_=d_xT[k + 1]).then_inc(sem["s_xt"], 16)

        # ================= PE stream =================
        if k == 0:
            pe.wait_ge(sem["s_pre"], PRE)
        # --- feed = i_feed @ Wfeed (skip at k=0: i_feed = 0 -> feed = 0) ---
        if k >= 1:
            pe.wait_ge(sem["rsA"], RS * cnt["A"])  # X_A(k-1) landed
            if k >= 2:
                pe.wait_ge(sem["s_feedt"], k - 1)  # ACT done reading ps_b1.feed(k-1)
            for j in range(HC):
                pe.matmul(
                    ps_b1[0:64, 256:384],
                    lhsT=land_A[:, prv, :][:, 64 * j:64 * j + 64],
                    rhs=wfeed[:, 128 * j:128 * j + 128],
                    start=(j == 0), stop=(j == HC - 1),
                ).then_maybe_inc((sem["s_feedp"], 1) if j == HC - 1 else None)

            # ACT: tanh(feed) -> feed_s (runs concurrent with PE gh/x MMs)
            act.wait_ge(sem["s_feedp"], k)  # k-th inc (steps 1..k)
            act.activation(feed_s, ps_b1[0:64, 256:384], AF.Tanh).then_inc(
                sem["s_feedt"], 1
            )

        # --- gh: h_prev @ Wh ---
        hsrc = land_H[:, 1, :] if k == 0 else land_H[:, prv, :]
        if k >= 1:
            pe.wait_ge(sem["s_zrdone"], k)   # ACT done reading ps_zr(k-1)
            pe.wait_ge(sem["s_gates"], k)    # DVE done reading ps_b1.xh/hh(k-1)
        for j in range(HC):
            lhs = hsrc[:, 64 * j:64 * j + 64]
            pe.matmul(ps_zr[0:64, 0:384], lhsT=lhs, rhs=wh[:, 384 * j:384 * j + 384],
                      start=(j == 0), stop=(j == HC - 1), skip_group_check=True)
        # --- gx x-part ---
        if k >= 1:
            pe.wait_ge(sem["s_xt"], 16 * k)
        for j in range(HC):
            lhs = xt[:, 64 * j:64 * j + 64]
            last = (k == 0) and (j == HC - 1)
            mm1 = pe.matmul(ps_zr[0:64, 0:256], lhsT=lhs,
                            rhs=wx[:, 384 * (8 + j):384 * (8 + j) + 256],
                            start=False, stop=last, skip_group_check=True)
            mm2 = pe.matmul(ps_b1[0:64, 0:128], lhsT=lhs,
                            rhs=wx[:, 384 * (8 + j) + 256:384 * (8 + j) + 384],
                            start=(j == 0), stop=last, skip_group_check=True)
            if j == HC - 1 and k == 0:
                mm1.then_inc(sem["s_zr"], 1)
                mm2.then_inc(sem["s_h2"], 1)
        # --- feedT transpose + X_F exchange (PE stayed dense through gh/x) ---
        if k >= 1:
            pe.wait_ge(sem["s_feedt"], k)
            pe.transpose(ps_tr[0:128, 0:64], feed_s, ident64).then_inc(sem["s_ftr"], 1)
            dve.wait_ge(sem["s_ftr"], k)
            if cnt["F"] >= (1 if strict_lsem else 2):
                dve.wait_ge(sem["lsF"], 16 * (cnt["F"] if strict_lsem else cnt["F"] - 1))
            dve.tensor_copy(send_F[:], ps_tr[:, 0:64]).then_inc(sem["s_fsend"], 1)
            fire("F", pF, "s_fsend", k)
        # --- gx feed-part (k>=1) ---
        if k >= 1:
            pe.wait_ge(sem["rsF"], RS * cnt["F"])  # X_F(k) landed
            for j in range(HC):
                lhs = land_F[:, 64 * j:64 * j + 64]
                last = j == HC - 1
                mm1 = pe.matmul(ps_zr[0:64, 0:256], lhsT=lhs,
                                rhs=wx[:, 384 * j:384 * j + 256],
                                start=False, stop=last, skip_group_check=True)
                mm2 = pe.matmul(ps_b1[0:64, 0:128], lhsT=lhs,
                                rhs=wx[:, 384 * j + 256:384 * j + 384],
                                start=False, stop=last, skip_group_check=True)
                if last:
                    mm1.then_inc(sem["s_zr"], 1)
                    mm2.then_inc(sem["s_h2"], 1)

        # ================= gates (ACT + DVE) =================
        act.wait_ge(sem["s_zr"], k + 1)
        act.activation(zb, ps_zr[0:64, 0:128], AF.Sigmoid).then_inc(sem["s_sigz"], 1)
        act.activation(rb, ps_zr[0:64, 128:256], AF.Sigmoid).then_inc(
            sem["s_zrdone"], 1
        )

        dve.wait_ge(sem["s_zrdone"], k + 1)
        dve.wait_ge(sem["s_h2"], k + 1)
        dve.tensor_mul(t1, rb, ps_zr[0:64, 256:384])
        dve.drain()
        dve.tensor_add(t2, ps_b1[0:64, 0:128], t1).then_inc(sem["s_t2"], 1)
        act.wait_ge(sem["s_t2"], k + 1)
        act.activation(hcand, t2, AF.Tanh).then_inc(sem["s_hcand"], 1)
        hprev = h0own if k == 0 else h_new[:, prv, :]
        dve.wait_ge(sem["s_hcand"], k + 1)
        dve.wait_ge(sem["s_sigz"], k + 1)
        dve.tensor_sub(db, hprev, hcand)
        dve.drain()
        dve.tensor_mul(zd[:], zb, db)
        dve.drain()
        dve.tensor_add(h_new[:, par, :], hcand, zd[:]).then_inc(sem["s_gates"], 1)

        # PE: transpose h_new -> ps_tr[64:128]; DVE: copy -> send_H; fire X_H
        pe.wait_ge(sem["s_gates"], k + 1)
        pe.transpose(ps_tr[0:128, 64:128], h_new[:, par, :], ident64).then_inc(
            sem["s_htr"], 1
        )
        dve.wait_ge(sem["s_htr"], k + 1)
        if cnt["H"] >= (1 if strict_lsem else 2):
            dve.wait_ge(sem["lsH"], 16 * (cnt["H"] if strict_lsem else cnt["H"] - 1))
        dve.tensor_copy(send_H[:], ps_tr[:, 64:128]).then_inc(sem["s_hsend"], 1)
        fire("H", pH, "s_hsend", k + 1)

        # ================= attention =================
        # sync: extract own-batch h_newT columns from land_H
        act.wait_ge(sem["rsH"], RS * cnt["H"])
        lh3 = land_H[:, par, :].rearrange("p (s c) -> p s c", c=64)
        act.dma_start(out=hsc[:].rearrange("p (s c) -> p s c", c=8),
                      in_=lh3[:, :, bass.ts(my_sc, 8)]).then_inc(sem["s_hscf"], 16)

        # PE: scores MMs (col-tiled, 2 rounds x 4 groups x 8 chunks; b = 2g + r)
        pe.wait_ge(sem["s_hscf"], 16 * (k + 1))
        for r in range(2):
            for j in range(HC):
                for g in range(4):
                    b = 2 * g + r
                    pe.matmul(
                        ps_sc[32 * g:32 * g + 1, 256 * r:256 * r + 256],
                        lhsT=hsc[:, 8 * j + b:8 * j + b + 1],
                        rhs=oencT[:, (j * BC + b) * TE:(j * BC + b) * TE + TE],
                        start=(j == 0), stop=(j == HC - 1),
                        tile_position=(0, 32 * g), skip_group_check=True,
                    ).then_maybe_inc(
                        (sem["s_sc"], 1) if (r == 1 and j == HC - 1 and g == 3) else None)

        # softmax reads scores PSUM directly (negated max via reduce negate flag)
        dve.wait_ge(sem["s_sc"], k + 1)
        dve.tensor_reduce(negmx_st[:],
                          ps_sc[:, 0:512].rearrange("p (r t) -> p r t", t=TE),
                          axis=AX.X, op=ALU.max, negate=True).then_inc(sem["s_negmx"], 1)
        act.wait_ge(sem["s_negmx"], k + 1)
        for r in range(2):
            act.activation(exp_st[:, 256 * r:256 * r + 256],
                           ps_sc[:, 256 * r:256 * r + 256], AF.Exp,
                           bias=negmx_st[:, r:r + 1],
                           accum_out=sume_st[:, r:r + 1]).then_maybe_inc(
                (sem["s_exp"], 1) if r == 1 else None)
        # normalization folded into the value-copy scale; recip runs off-chain
        dve.wait_ge(sem["s_exp"], k + 1)
        dve.reciprocal(rsum_st[:], sume_st[:]).then_inc(sem["s_attn"], 1)

        # sync: gather UNNORMALIZED exp rows (b = 2g + r) -> attn [8, 256]
        sy.wait_ge(sem["s_exp"], k + 1)
        act.wait_ge(sem["s_exp"], k + 1)
        for r, eng in ((0, sy), (1, act)):
            eng.dma_start(out=attn[r:8:2, :],
                          in_=exp_st[0:128:32, 256 * r:256 * r + 256]).then_inc(
                sem["s_attng"], 16)

        # PE: attnT transposes; DVE: cast to bf16
        pe.wait_ge(sem["s_attng"], 32 * (k + 1))
        if k >= 1:
            pe.wait_ge(sem["s_attnT"], k)  # DVE done reading ps_tr[128:144] (k-1)
        for tc in range(2):
            pe.transpose(ps_tr[0:128, 128 + 8 * tc:136 + 8 * tc],
                         attn[:, 128 * tc:128 * tc + 128], ident8).then_maybe_inc(
                (sem["s_attr"], 1) if tc == 1 else None)
        dve.wait_ge(sem["s_attr"], k + 1)
        dve.tensor_copy(attnT[:], ps_tr[:, 128:144]).then_inc(sem["s_attnT"], 1)

        # PE: value MMs (col-tiled)
        pe.wait_ge(sem["s_attnT"], k + 1)
        if k >= 1:
            pe.wait_ge(sem["s_vx"], 32 * k)  # sync done reading ps_v(k-1)
        for r in range(2):
            psv = ps_v0 if r == 0 else ps_v1
            for tc in range(2):
                for g in range(4):
                    b = 2 * g + r
                    for hf in range(2):
                        pe.matmul(
                            psv[32 * g:32 * g + 1, 512 * hf:512 * hf + 512],
                            lhsT=attnT[:, 8 * tc + b:8 * tc + b + 1],
                            rhs=oenc[:, (b * 2 + tc) * 1024 + 512 * hf:
                                     (b * 2 + tc) * 1024 + 512 * hf + 512],
                            start=(tc == 0), stop=(tc == 1),
                            tile_position=(0, 32 * g), skip_group_check=True,
                        ).then_maybe_inc(
                            (sem["s_v"], 1)
                            if (r == 1 and tc == 1 and g == 3 and hf == 1) else None)

        # ACT: bulk copy value psum -> sbuf (scratch reused; wait attn gather done)
        act.wait_ge(sem["s_attng"], 32 * (k + 1))
        act.wait_ge(sem["s_attn"], k + 1)  # rsum ready (DVE recip)
        act.wait_ge(sem["s_v"], k + 1)
        act.activation(v_st[:, 0:1024], ps_v0[:], AF.Copy, scale=rsum_st[:, 0:1])
        act.activation(v_st[:, 1024:2048], ps_v1[:], AF.Copy,
                       scale=rsum_st[:, 1:2]).then_inc(sem["s_vst"], 1)
        sy.wait_ge(sem["s_vst"], k + 1)
        act.wait_ge(sem["s_vst"], k + 1)
        for r, eng in ((0, sy), (1, act)):
            eng.dma_start(out=value[r:8:2, :],
                          in_=v_st[0:128:32, 1024 * r:1024 * r + 1024]).then_inc(
                sem["s_vx"], 16)

        # PE: valueT transposes -> ps_tr[144:208]; DVE: copy -> send_V; fire X_V
        pe.wait_ge(sem["s_vx"], 32 * (k + 1))
        if k >= 1:
            pe.wait_ge(sem["s_vsend"], k)  # DVE done reading ps_tr[144:208] (k-1)
        for j in range(HC):
            pe.transpose(ps_tr[0:128, 144 + 8 * j:152 + 8 * j],
                         value[:, 128 * j:128 * j + 128], ident8).then_maybe_inc(
                (sem["s_vtr"], 1) if j == HC - 1 else None)
        dve.wait_ge(sem["s_vtr"], k + 1)
        if cnt["V"] >= (1 if strict_lsem else 2):
            dve.wait_ge(sem["lsV"], 16 * (cnt["V"] if strict_lsem else cnt["V"] - 1))
        dve.tensor_copy(send_V[:], ps_tr[:, 144:208]).then_inc(sem["s_vsend"], 1)
        fire("V", pV, "s_vsend", k + 1)

        # DVE: repack land_V (snd, hc, bl) -> (hc, snd, bl) so Watt lhsT tiles are contiguous
        dve.wait_ge(sem["rsV"], RS * cnt["V"])
        dve.wait_ge(sem["s_vx"], 32 * (k + 1))  # scratch free (v gather done)
        lv3 = land_V[:].rearrange("p (s hc bl) -> p hc s bl", hc=HC, bl=8)
        dve.tensor_copy(wv_re, lv3).then_inc(sem["s_wvre"], 1)

        # PE: Watt MMs: h_att_slice = tanh(cat[value, h_new] @ Watt[:, S_c])
        pe.wait_ge(sem["s_wvre"], k + 1)
        if k >= 1:
            pe.wait_ge(sem["s_hatt"], k)  # ACT done reading ps_b1.att(k-1)
        for j in range(16):
            if j < HC:
                lhs = scratch[:, 64 * j:64 * j + 64]
            else:
                lhs = land_H[:, par, :][:, 64 * (j - 8):64 * (j - 8) + 64]
            pe.matmul(ps_b1[0:64, 384:512], lhsT=lhs,
                      rhs=watt[:, 128 * j:128 * j + 128],
                      start=(j == 0), stop=(j == 15),
                      skip_group_check=True).then_maybe_inc(
                (sem["s_att"], 1) if j == 15 else None)

        # ACT: tanh -> h_att
        act.wait_ge(sem["s_att"], k + 1)
        act.activation(gp5[64:128, :], ps_b1[0:64, 384:512], AF.Tanh).then_inc(sem["s_hatt"], 1)

        # sync: store output slice
        sy.wait_ge(sem["s_hatt"], k + 1)
        sy.dma_start(out=d_out[:, k, :], in_=gp5[64:128, :]).then_inc(sem["s_out"], 16)

        # PE: h_attT transpose; DVE: copy -> send_A; fire X_A
        if k < steps - 1:
            pe.wait_ge(sem["s_hatt"], k + 1)
            pe.transpose(ps_tr[0:128, 208:272], gp5[64:128, :], ident64b).then_inc(sem["s_atr"], 1)
            dve.wait_ge(sem["s_atr"], k + 1)
            if cnt["A"] >= (1 if strict_lsem else 2):
                dve.wait_ge(sem["lsA"], 16 * (cnt["A"] if strict_lsem else cnt["A"] - 1))
            dve.tensor_copy(send_A[:], ps_tr[:, 208:272]).then_inc(sem["s_asend"], 1)
            fire("A", pA, "s_asend", cnt["A"] + 1)

    # final: make sure all stores landed before kernel end
    sy.wait_ge(sem["s_out"], 16 * steps)
    nc.all_core_barrier()
    return nc


_NC_CACHE = {}


def _get_nc(steps=T, strict_lsem=False, loopback=False):
    key = (steps, strict_lsem, loopback)
    if key not in _NC_CACHE:
        nc = bacc.Bacc("TRN2", target_bir_lowering=False, debug=False,
                       num_devices=NCORES)
        build(nc, steps, strict_lsem=strict_lsem, loopback=loopback)
        nc.compile()
        _NC_CACHE[key] = nc
    return _NC_CACHE[key]


def make_in_maps(x, o_enc, h_enc, Wfeed, bfeed, Wx, Wh, bxi, bhr, Watt, batt):
    """Host-side sharding/layout prep. Biases are zeros in this problem and are
    folded out (asserted)."""
    for b_ in (bfeed, bxi, bhr, batt):
        assert np.abs(np.asarray(b_)).max() == 0.0
    x = np.asarray(x, np.float32)
    o_enc = np.asarray(o_enc, np.float32)
    h_enc = np.asarray(h_enc, np.float32)
    Wfeed = np.asarray(Wfeed, np.float32)
    Wx = np.asarray(Wx, np.float32)
    Wh = np.asarray(Wh, np.float32)
    Watt = np.asarray(Watt, np.float32)

    xT = np.ascontiguousarray(
        x.transpose(1, 2, 0).reshape(T, HC, 128, B).transpose(0, 2, 1, 3)
        .reshape(T, 128, HC * B))                             # [T, 128p, (hc b)]
    h0T = np.ascontiguousarray(h_enc.T)                       # [H, B]
    ident = np.vstack([np.eye(64, dtype=np.float32)] * 2)
    maps = []
    for c in range(NCORES):
        S = slice(128 * c, 128 * c + 128)
        wx_c = np.concatenate([Wx[:, S], Wx[:, 1024:][:, S], Wx[:, 2048:][:, S]],
                              axis=1)                          # [2048, 384]
        wh_c = np.concatenate([Wh[:, S], Wh[:, 1024:][:, S], Wh[:, 2048:][:, S]],
                              axis=1)                          # [1024, 384]
        ob = o_enc[BC * c:BC * c + BC]                         # [8, 256, 1024]
        oT = np.ascontiguousarray(ob.transpose(2, 0, 1)).reshape(H, BC * TE)
        oe = np.ascontiguousarray(ob.reshape(BC * TE, H).astype(np.float32))
        maps.append({
            "wfeed": np.ascontiguousarray(Wfeed[:, S]),
            "wx": np.ascontiguousarray(wx_c),
            "wh": np.ascontiguousarray(wh_c),
            "watt": np.ascontiguousarray(Watt[:, S]),
            "oencT": oT,
            "oenc": oe,
            "xT": xT,
            "h0T": h0T,
            "h0own": np.ascontiguousarray(h_enc[:, S]),
            "ident": ident,
        })
    return maps


def kernel(**inputs):
    from concourse.bass_utils import run_bass_kernel_spmd

    nc = _get_nc(T)
    in_maps = make_in_maps(**inputs)
    res = run_bass_kernel_spmd(nc, in_maps, list(range(NCORES)))
    out = np.concatenate(
        [np.asarray(res.results[c]["out"], np.float32) for c in range(NCORES)], axis=2
    )
    return out

